# revision 2
# baseline (speedup 1.0000x reference)
"""Trainium2 Bass kernel for nn_DensityGrid (morton-scatter density grid update).

Strategy
--------
The reference scatters MLP outputs by morton code:  tmp[morton(i)] = mlp(x_i).
Morton encoding of the full 128^3 grid is a fixed, input-independent
permutation, so the scatter is folded into HOST-side packing: for every output
cell j (morton order) we gather jitter[morton_inv(j)] and precompute the exact
fp32 MLP input x on the host, packed in the exact SBUF layout the kernel
streams.  The device kernel is then a pure streaming pipeline:

  per core (1/8 contiguous slab of the cell dim, = one 64^3 coordinate octant):
    xp  [4 lvl, 4 supertile, 4 quarters, 12, 8*512]  ->  (block-diag W1, 4
    cells/column, K=12) fp32 matmul -> Relu(+b1) on ACT -> second matmul with
    a shifted-slice block-diag W2 accumulating 32 matmuls into one PSUM bank
    -> softplus composed as Relu(d+b2) + Ln(Exp(-|d+b2|)+1) -> fused
    EMA max(0.95*dens, D) with per-partition running sums -> AllReduce of the
    scalar sum across 8 cores -> threshold -> bitfield pack (fused
    is_gt*weight + segmented reduce) -> uint8 slab.

All matmuls run in fp32 (4 cyc/row) because the bitfield output compares
values against a global-mean threshold: any low-precision matmul flips
near-threshold bits vs. the fp32 reference.
"""
import os
import numpy as np

import concourse.bacc as bacc
import concourse.tile as tile
import concourse.mybir as mybir
from concourse import bass_utils, bass_isa

dt = mybir.dt
AF = mybir.ActivationFunctionType
ALU = mybir.AluOpType

N_CORES = 8
RES = 128
CASC = 4
N_CELLS = RES ** 3                    # 2097152
SLAB = N_CELLS // N_CORES             # 262144 cells per core (one 64^3 octant)
N_ST = CASC * 4                       # 16 supertiles per core (4 lvl x 4 T)
DECAY = np.float32(0.95)
INV_TOTAL = np.float32(1.0 / (CASC * N_CELLS))   # == 2^-23, exact
DENS_THRESH = 2.0

_CACHE = {}
LAST_RESULT = None                    # BassKernelResults of the latest run


# ---------------------------------------------------------------- host packing
def _compact3(v):
    """Inverse of instant-ngp _expand_bits: gather bits 0,3,6,... of v."""
    v = v.astype(np.uint32) & np.uint32(0x49249249)
    v = (v | (v >> np.uint32(2))) & np.uint32(0xC30C30C3)
    v = (v | (v >> np.uint32(4))) & np.uint32(0x0F00F00F)
    v = (v | (v >> np.uint32(8))) & np.uint32(0xFF0000FF)
    v = (v | (v >> np.uint32(16))) & np.uint32(0x000003FF)
    return v


def _decode_tables():
    """Per output cell j (morton order): source cell index and float coords."""
    if "dec" not in _CACHE:
        j = np.arange(N_CELLS, dtype=np.uint32)
        x = _compact3(j)
        y = _compact3(j >> np.uint32(1))
        z = _compact3(j >> np.uint32(2))
        i_cell = (x.astype(np.int64) << 14) | (y.astype(np.int64) << 7) | z
        cf = np.stack([x, y, z], axis=1).astype(np.float32)   # [N_CELLS, 3]
        _CACHE["dec"] = (i_cell, cf)
    return _CACHE["dec"]


def _pack_core_inputs(density_grid, jitter, W1, b1, W2, b2, core):
    i_cell, cf = _decode_tables()
    lo, hi = core * SLAB, (core + 1) * SLAB
    idx = i_cell[lo:hi]
    cfs = cf[lo:hi]                                           # [SLAB, 3]

    xp = np.empty((CASC, 4, 4, 12, 8 * 512), np.float32)
    for lvl in range(CASC):
        # exact fp32 replica of the reference ops
        t = (cfs + jitter[lvl, idx]) / np.float32(RES)
        xv = (t - np.float32(0.5)) * np.float32(2.0 ** lvl)   # [SLAB, 3]
        # cell j = T*65536 + P*512 + s, P = 4*m + g, m = mo*8 + mi
        v = xv.reshape(4, 4, 8, 4, 512, 3)                    # T, mo, mi, g, s, c
        xp[lvl] = (
            v.transpose(0, 1, 3, 5, 2, 4)                     # T, mo, g, c, mi, s
            .reshape(4, 4, 12, 8 * 512)
        )
    dens = density_grid[:, lo:hi].reshape(CASC, 4, 128, 512).copy()
    return xp, dens


def _shared_inputs(W1, b1, W2, b2):
    w1bd = np.zeros((12, 128), np.float32)
    gmat = np.zeros((128, 252), np.float32)
    for g in range(4):
        w1bd[g * 3:(g + 1) * 3, g * 32:(g + 1) * 32] = W1
        gmat[g * 32:(g + 1) * 32, 124 + g] = W2[:, 0]
    b1v = np.tile(np.asarray(b1, np.float32), 4).reshape(128, 1).copy()
    b2v = np.full((128, 1), np.asarray(b2, np.float32).reshape(-1)[0], np.float32)
    wpat = np.broadcast_to(
        np.tile((1 << np.arange(8)).astype(np.float32), 64), (128, 512)
    ).copy()
    return {"w1bd": w1bd, "gmat": gmat, "b1v": b1v, "b2v": b2v, "wpat": wpat}


# ---------------------------------------------------------------- bass program
def _build_nc():
    nc = bacc.Bacc("TRN2", num_devices=N_CORES)
    xp_t = nc.dram_tensor("xp", [CASC, 4, 4, 12, 8 * 512], dt.float32,
                          kind="ExternalInput")
    dens_t = nc.dram_tensor("dens", [CASC, 4, 128, 512], dt.float32,
                            kind="ExternalInput")
    w1_t = nc.dram_tensor("w1bd", [12, 128], dt.float32, kind="ExternalInput")
    g_t = nc.dram_tensor("gmat", [128, 252], dt.float32, kind="ExternalInput")
    b1_t = nc.dram_tensor("b1v", [128, 1], dt.float32, kind="ExternalInput")
    b2_t = nc.dram_tensor("b2v", [128, 1], dt.float32, kind="ExternalInput")
    wp_t = nc.dram_tensor("wpat", [128, 512], dt.float32, kind="ExternalInput")

    new_t = nc.dram_tensor("new_out", [CASC, 4, 128, 512], dt.float32,
                           kind="ExternalOutput")
    bf_t = nc.dram_tensor("bf_out", [CASC, 4, 128, 64], dt.uint8,
                          kind="ExternalOutput")
    mean_t = nc.dram_tensor("mean_out", [1, 1], dt.float32,
                            kind="ExternalOutput")

    with tile.TileContext(nc) as tc:
        with (
            tc.tile_pool(name="const", bufs=1) as cpool,
            tc.tile_pool(name="xs", bufs=3) as xpool,
            tc.tile_pool(name="work", bufs=3) as wpool,
            tc.tile_pool(name="keep", bufs=1) as kpool,
            tc.tile_pool(name="pa", bufs=3, space="PSUM") as papool,
            tc.tile_pool(name="pd", bufs=2, space="PSUM") as pdpool,
            tc.tile_pool(name="dram", bufs=1, space="DRAM") as drpool,
        ):
            w1 = cpool.tile([12, 128], dt.float32)
            gm = cpool.tile([128, 252], dt.float32)
            b1 = cpool.tile([128, 1], dt.float32)
            b2 = cpool.tile([128, 1], dt.float32)
            wp = cpool.tile([128, 512], dt.float32)
            for sb, src in [(w1, w1_t), (gm, g_t), (b1, b1_t), (b2, b2_t),
                            (wp, wp_t)]:
                nc.sync.dma_start(sb[:], src[:])

            new_all = kpool.tile([128, N_ST * 512], dt.float32)
            stats = kpool.tile([128, N_ST], dt.float32)

            for st in range(N_ST):
                lvl, T = divmod(st, 4)
                dn = wpool.tile([128, 512], dt.float32, tag="dn")
                nc.sync.dma_start(dn[:], dens_t[lvl, T])
                dacc = pdpool.tile([128, 512], dt.float32, tag="dacc")
                pend = None
                for mo in range(4):
                    xs = xpool.tile([12, 8 * 512], dt.float32, tag="xs")
                    nc.sync.dma_start(xs[:], xp_t[lvl, T, mo])
                    for mi in range(8):
                        m = mo * 8 + mi
                        aps = papool.tile([128, 512], dt.float32, tag="aps")
                        nc.tensor.matmul(aps[:], w1[:],
                                         xs[:, mi * 512:(mi + 1) * 512],
                                         start=True, stop=True)
                        h = wpool.tile([128, 512], dt.float32, tag="h")
                        nc.scalar.activation(h[:], aps[:], AF.Relu,
                                             bias=b1[:], scale=1.0)
                        if pend is not None:
                            pm, ph = pend
                            nc.tensor.matmul(
                                dacc[:], gm[:, 124 - 4 * pm:252 - 4 * pm],
                                ph[:], start=(pm == 0), stop=False)
                        pend = (m, h)
                pm, ph = pend
                nc.tensor.matmul(dacc[:], gm[:, 124 - 4 * pm:252 - 4 * pm],
                                 ph[:], start=False, stop=True)

                # softplus(d + b2) = relu(d+b2) + ln(exp(-|d+b2|) + 1)
                y = wpool.tile([128, 512], dt.float32, tag="y")
                nc.scalar.activation(y[:], dacc[:], AF.Abs, bias=b2[:],
                                     scale=1.0)
                nc.scalar.activation(y[:], y[:], AF.Exp, bias=0.0, scale=-1.0)
                nc.scalar.activation(y[:], y[:], AF.Ln, bias=1.0, scale=1.0)
                mx = wpool.tile([128, 512], dt.float32, tag="mx")
                nc.scalar.activation(mx[:], dacc[:], AF.Relu, bias=b2[:],
                                     scale=1.0)
                D = wpool.tile([128, 512], dt.float32, tag="D")
                nc.vector.tensor_add(D[:], mx[:], y[:])

                # EMA + per-partition sum accumulation
                new_sl = new_all[:, st * 512:(st + 1) * 512]
                nc.vector.scalar_tensor_tensor(
                    new_sl, dn[:], float(DECAY), D[:], op0=ALU.mult, op1=ALU.max,
                    accum_out=stats[:, st:st + 1])
                nc.sync.dma_start(new_t[lvl, T], new_sl)

            # ---- global mean via AllReduce
            loc = kpool.tile([128, 1], dt.float32)
            nc.vector.tensor_reduce(loc[:], stats[:],
                                    axis=mybir.AxisListType.X, op=ALU.add)
            red = kpool.tile([128, 1], dt.float32)
            nc.gpsimd.partition_all_reduce(red[:], loc[:], 128,
                                           bass_isa.ReduceOp.add)
            cc_in = drpool.tile([1, 1], dt.float32)
            cc_out = drpool.tile([1, 1], dt.float32, addr_space="Shared")
            nc.sync.dma_start(cc_in[:], red[0:1, :])
            nc.gpsimd.collective_compute(
                "AllReduce", ALU.add,
                replica_groups=[list(range(N_CORES))],
                ins=[cc_in[:]], outs=[cc_out[:]])
            tot = kpool.tile([1, 1], dt.float32)
            nc.sync.dma_start(tot[:], cc_out[:])
            mean11 = kpool.tile([1, 1], dt.float32)
            nc.vector.tensor_scalar_mul(mean11[:], tot[:], float(INV_TOTAL))
            nc.sync.dma_start(mean_t[:], mean11[:])
            th11 = kpool.tile([1, 1], dt.float32)
            nc.vector.tensor_scalar_min(th11[:], mean11[:], DENS_THRESH)
            thb = kpool.tile([128, 1], dt.float32)
            nc.gpsimd.partition_broadcast(thb[:], th11[:])

            # ---- bitfield pack
            for st in range(N_ST):
                lvl, T = divmod(st, 4)
                wt = wpool.tile([128, 512], dt.float32, tag="wt")
                nc.vector.scalar_tensor_tensor(
                    wt[:], new_all[:, st * 512:(st + 1) * 512], thb[:], wp[:],
                    op0=ALU.is_gt, op1=ALU.mult)
                bf32 = wpool.tile([128, 64], dt.float32, tag="bf32")
                nc.vector.tensor_reduce(
                    bf32[:], wt[:].rearrange("p (a b) -> p a b", b=8),
                    axis=mybir.AxisListType.X, op=ALU.add)
                bfu8 = wpool.tile([128, 64], dt.uint8, tag="bfu8")
                nc.vector.tensor_copy(bfu8[:], bf32[:])
                nc.sync.dma_start(bf_t[lvl, T], bfu8[:])

    nc.finalize()
    return nc


# ---------------------------------------------------------------- entry point
def kernel(density_grid, jitter, W1, b1, W2, b2):
    global LAST_RESULT
    density_grid = np.ascontiguousarray(density_grid, np.float32)
    jitter = np.ascontiguousarray(jitter, np.float32)
    W1 = np.asarray(W1, np.float32)
    b1 = np.asarray(b1, np.float32)
    W2 = np.asarray(W2, np.float32)
    b2 = np.asarray(b2, np.float32)

    shared = _shared_inputs(W1, b1, W2, b2)
    in_maps = []
    for k in range(N_CORES):
        xp, dens = _pack_core_inputs(density_grid, jitter, W1, b1, W2, b2, k)
        in_maps.append({"xp": xp, "dens": dens, **shared})

    if "nc" not in _CACHE:
        _CACHE["nc"] = _build_nc()
    nc = _CACHE["nc"]

    trace = bool(int(os.environ.get("DG_TRACE", "0")))
    res = bass_utils.run_bass_kernel_spmd(
        nc, in_maps, core_ids=list(range(N_CORES)), trace=trace)
    LAST_RESULT = res
    outs = res.results

    new_grid = np.empty((CASC, N_CELLS), np.float32)
    bf = np.empty((CASC, N_CELLS // 8), np.uint8)
    for k in range(N_CORES):
        new_grid[:, k * SLAB:(k + 1) * SLAB] = \
            outs[k]["new_out"].reshape(CASC, SLAB)
        bf[:, k * SLAB // 8:(k + 1) * SLAB // 8] = \
            outs[k]["bf_out"].reshape(CASC, SLAB // 8)
    mean_density = outs[0]["mean_out"].reshape(())[()]
    return new_grid, np.float32(mean_density), bf.reshape(-1)


# revision 4
# speedup vs baseline: 1.6659x; 1.6659x over previous
"""Trainium2 Bass kernel for nn_DensityGrid (morton-scatter density grid update).

Strategy
--------
The reference scatters MLP outputs by morton code:  tmp[morton(i)] = mlp(x_i).
Morton encoding of the full 128^3 grid is a fixed, input-independent
permutation, so the scatter is folded into HOST-side packing: for every output
cell j (morton order) we gather jitter[morton_inv(j)] and precompute the exact
fp32 MLP input x on the host, packed in the exact SBUF layout the kernel
streams.  The device kernel is then a pure streaming pipeline:

  per core (1/8 contiguous slab of the cell dim, = one 64^3 coordinate octant):
    xp  [4 lvl, 4 supertile, 4 quarters, 12, 8*512]  ->  (block-diag W1, 4
    cells/column, K=12) fp32 matmul -> Relu(+b1) on ACT -> second matmul with
    a shifted-slice block-diag W2 accumulating 32 matmuls into one PSUM bank
    -> softplus composed as Relu(d+b2) + Ln(Exp(-|d+b2|)+1) -> fused
    EMA max(0.95*dens, D) with per-partition running sums -> AllReduce of the
    scalar sum across 8 cores -> threshold -> bitfield pack (fused
    is_gt*weight + segmented reduce) -> uint8 slab.

All matmuls run in fp32 (4 cyc/row) because the bitfield output compares
values against a global-mean threshold: any low-precision matmul flips
near-threshold bits vs. the fp32 reference.
"""
import os
import numpy as np

import concourse.bacc as bacc
import concourse.tile as tile
import concourse.mybir as mybir
from concourse import bass_utils, bass_isa

dt = mybir.dt
AF = mybir.ActivationFunctionType
ALU = mybir.AluOpType

N_CORES = 8
RES = 128
CASC = 4
N_CELLS = RES ** 3                    # 2097152
SLAB = N_CELLS // N_CORES             # 262144 cells per core (one 64^3 octant)
N_ST = CASC * 4                       # 16 supertiles per core (4 lvl x 4 T)
DECAY = np.float32(0.95)
INV_TOTAL = np.float32(1.0 / (CASC * N_CELLS))   # == 2^-23, exact
DENS_THRESH = 2.0

_CACHE = {}
LAST_RESULT = None                    # BassKernelResults of the latest run


# ---------------------------------------------------------------- host packing
def _compact3(v):
    """Inverse of instant-ngp _expand_bits: gather bits 0,3,6,... of v."""
    v = v.astype(np.uint32) & np.uint32(0x49249249)
    v = (v | (v >> np.uint32(2))) & np.uint32(0xC30C30C3)
    v = (v | (v >> np.uint32(4))) & np.uint32(0x0F00F00F)
    v = (v | (v >> np.uint32(8))) & np.uint32(0xFF0000FF)
    v = (v | (v >> np.uint32(16))) & np.uint32(0x000003FF)
    return v


def _decode_tables():
    """Per output cell j (morton order): source cell index and float coords."""
    if "dec" not in _CACHE:
        j = np.arange(N_CELLS, dtype=np.uint32)
        x = _compact3(j)
        y = _compact3(j >> np.uint32(1))
        z = _compact3(j >> np.uint32(2))
        i_cell = (x.astype(np.int64) << 14) | (y.astype(np.int64) << 7) | z
        cf = np.stack([x, y, z], axis=1).astype(np.float32)   # [N_CELLS, 3]
        _CACHE["dec"] = (i_cell, cf)
    return _CACHE["dec"]


def _pack_core_inputs(density_grid, jitter, W1, b1, W2, b2, core):
    i_cell, cf = _decode_tables()
    lo, hi = core * SLAB, (core + 1) * SLAB
    idx = i_cell[lo:hi]
    cfs = cf[lo:hi]                                           # [SLAB, 3]

    xp = np.empty((CASC, 4, 4, 12, 8 * 512), np.float32)
    for lvl in range(CASC):
        # exact fp32 replica of the reference ops
        t = (cfs + jitter[lvl, idx]) / np.float32(RES)
        xv = (t - np.float32(0.5)) * np.float32(2.0 ** lvl)   # [SLAB, 3]
        # cell j = T*65536 + P*512 + s, P = 4*m + g, m = mo*8 + mi
        v = xv.reshape(4, 4, 8, 4, 512, 3)                    # T, mo, mi, g, s, c
        xp[lvl] = (
            v.transpose(0, 1, 3, 5, 2, 4)                     # T, mo, g, c, mi, s
            .reshape(4, 4, 12, 8 * 512)
        )
    dens = density_grid[:, lo:hi].reshape(CASC, 4, 128, 512).copy()
    return xp, dens


def _shared_inputs(W1, b1, W2, b2):
    w1bd = np.zeros((12, 128), np.float32)
    gmat = np.zeros((128, 252), np.float32)
    for g in range(4):
        w1bd[g * 3:(g + 1) * 3, g * 32:(g + 1) * 32] = W1
        gmat[g * 32:(g + 1) * 32, 124 + g] = W2[:, 0]
    b1v = np.tile(np.asarray(b1, np.float32), 4).reshape(128, 1).copy()
    b2v = np.full((128, 1), np.asarray(b2, np.float32).reshape(-1)[0], np.float32)
    wpat = np.broadcast_to(
        np.tile((1 << np.arange(8)).astype(np.float32), 64), (128, 512)
    ).copy()
    return {"w1bd": w1bd, "gmat": gmat, "b1v": b1v, "b2v": b2v, "wpat": wpat}


# ---------------------------------------------------------------- bass program
def _build_nc(reps=1):
    nc = bacc.Bacc("TRN2", num_devices=N_CORES)
    xp_t = nc.dram_tensor("xp", [CASC, 4, 4, 12, 8 * 512], dt.float32,
                          kind="ExternalInput")
    dens_t = nc.dram_tensor("dens", [CASC, 4, 128, 512], dt.float32,
                            kind="ExternalInput")
    w1_t = nc.dram_tensor("w1bd", [12, 128], dt.float32, kind="ExternalInput")
    g_t = nc.dram_tensor("gmat", [128, 252], dt.float32, kind="ExternalInput")
    b1_t = nc.dram_tensor("b1v", [128, 1], dt.float32, kind="ExternalInput")
    b2_t = nc.dram_tensor("b2v", [128, 1], dt.float32, kind="ExternalInput")
    wp_t = nc.dram_tensor("wpat", [128, 512], dt.float32, kind="ExternalInput")

    new_t = nc.dram_tensor("new_out", [CASC, 4, 128, 512], dt.float32,
                           kind="ExternalOutput")
    bf_t = nc.dram_tensor("bf_out", [CASC, 4, 128, 64], dt.uint8,
                          kind="ExternalOutput")
    mean_t = nc.dram_tensor("mean_out", [1, 1], dt.float32,
                            kind="ExternalOutput")

    with tile.TileContext(nc) as tc:
        with (
            tc.tile_pool(name="const", bufs=1) as cpool,
            tc.tile_pool(name="xs", bufs=3) as xpool,
            tc.tile_pool(name="work", bufs=3) as wpool,
            tc.tile_pool(name="keep", bufs=1) as kpool,
            tc.tile_pool(name="pa", bufs=3, space="PSUM") as papool,
            tc.tile_pool(name="pd", bufs=2, space="PSUM") as pdpool,
            tc.tile_pool(name="dram", bufs=1, space="DRAM") as drpool,
        ):
            w1 = cpool.tile([12, 128], dt.float32)
            gm = cpool.tile([128, 252], dt.float32)
            b1 = cpool.tile([128, 1], dt.float32)
            b2 = cpool.tile([128, 1], dt.float32)
            wp = cpool.tile([128, 512], dt.float32)
            for sb, src in [(w1, w1_t), (gm, g_t), (b1, b1_t), (b2, b2_t),
                            (wp, wp_t)]:
                nc.sync.dma_start(sb[:], src[:])

            new_all = kpool.tile([128, N_ST * 512], dt.float32)
            stats = kpool.tile([128, N_ST], dt.float32)

            for st in range(N_ST * reps):
                st = st % N_ST
                lvl, T = divmod(st, 4)
                dn = wpool.tile([128, 512], dt.float32, tag="dn")
                nc.sync.dma_start(dn[:], dens_t[lvl, T])
                dacc = pdpool.tile([128, 512], dt.float32, tag="dacc")
                pend = None
                for mo in range(4):
                    xs = xpool.tile([12, 8 * 512], dt.float32, tag="xs")
                    nc.sync.dma_start(xs[:], xp_t[lvl, T, mo])
                    for mi in range(8):
                        m = mo * 8 + mi
                        aps = papool.tile([128, 512], dt.float32, tag="aps")
                        nc.tensor.matmul(aps[:], w1[:],
                                         xs[:, mi * 512:(mi + 1) * 512],
                                         start=True, stop=True)
                        h = wpool.tile([128, 512], dt.float32, tag="h")
                        nc.scalar.activation(h[:], aps[:], AF.Relu,
                                             bias=b1[:], scale=1.0)
                        if pend is not None:
                            pm, ph = pend
                            nc.tensor.matmul(
                                dacc[:], gm[:, 124 - 4 * pm:252 - 4 * pm],
                                ph[:], start=(pm == 0), stop=False)
                        pend = (m, h)
                pm, ph = pend
                nc.tensor.matmul(dacc[:], gm[:, 124 - 4 * pm:252 - 4 * pm],
                                 ph[:], start=False, stop=True)

                # softplus(d + b2) = relu(d+b2) + ln(exp(-|d+b2|) + 1)
                y = wpool.tile([128, 512], dt.float32, tag="y")
                nc.scalar.activation(y[:], dacc[:], AF.Abs, bias=b2[:],
                                     scale=1.0)
                nc.scalar.activation(y[:], y[:], AF.Exp, bias=0.0, scale=-1.0)
                nc.scalar.activation(y[:], y[:], AF.Ln, bias=1.0, scale=1.0)
                mx = wpool.tile([128, 512], dt.float32, tag="mx")
                nc.scalar.activation(mx[:], dacc[:], AF.Relu, bias=b2[:],
                                     scale=1.0)
                D = wpool.tile([128, 512], dt.float32, tag="D")
                nc.vector.tensor_add(D[:], mx[:], y[:])

                # EMA + per-partition sum accumulation
                new_sl = new_all[:, st * 512:(st + 1) * 512]
                nc.vector.scalar_tensor_tensor(
                    new_sl, dn[:], float(DECAY), D[:], op0=ALU.mult, op1=ALU.max,
                    accum_out=stats[:, st:st + 1])
                nc.sync.dma_start(new_t[lvl, T], new_sl)

            # ---- global mean via AllReduce
            loc = kpool.tile([128, 1], dt.float32)
            nc.vector.tensor_reduce(loc[:], stats[:],
                                    axis=mybir.AxisListType.X, op=ALU.add)
            red = kpool.tile([128, 1], dt.float32)
            nc.gpsimd.partition_all_reduce(red[:], loc[:], 128,
                                           bass_isa.ReduceOp.add)
            cc_in = drpool.tile([1, 1], dt.float32)
            cc_out = drpool.tile([1, 1], dt.float32, addr_space="Shared")
            nc.sync.dma_start(cc_in[:], red[0:1, :])
            nc.gpsimd.collective_compute(
                "AllReduce", ALU.add,
                replica_groups=[list(range(N_CORES))],
                ins=[cc_in[:]], outs=[cc_out[:]])
            tot = kpool.tile([1, 1], dt.float32)
            nc.sync.dma_start(tot[:], cc_out[:])
            mean11 = kpool.tile([1, 1], dt.float32)
            nc.vector.tensor_scalar_mul(mean11[:], tot[:], float(INV_TOTAL))
            nc.sync.dma_start(mean_t[:], mean11[:])
            th11 = kpool.tile([1, 1], dt.float32)
            nc.vector.tensor_scalar_min(th11[:], mean11[:], DENS_THRESH)
            thb = kpool.tile([128, 1], dt.float32)
            nc.gpsimd.partition_broadcast(thb[:], th11[:])

            # ---- bitfield pack
            for st in range(N_ST):
                lvl, T = divmod(st, 4)
                wt = wpool.tile([128, 512], dt.float32, tag="wt")
                nc.vector.scalar_tensor_tensor(
                    wt[:], new_all[:, st * 512:(st + 1) * 512], thb[:], wp[:],
                    op0=ALU.is_gt, op1=ALU.mult)
                bf32 = wpool.tile([128, 64], dt.float32, tag="bf32")
                nc.vector.tensor_reduce(
                    bf32[:], wt[:].rearrange("p (a b) -> p a b", b=8),
                    axis=mybir.AxisListType.X, op=ALU.add)
                bfu8 = wpool.tile([128, 64], dt.uint8, tag="bfu8")
                nc.vector.tensor_copy(bfu8[:], bf32[:])
                nc.sync.dma_start(bf_t[lvl, T], bfu8[:])

    nc.finalize()
    return nc


# ---------------------------------------------------------------- entry point
def kernel(density_grid, jitter, W1, b1, W2, b2):
    global LAST_RESULT
    density_grid = np.ascontiguousarray(density_grid, np.float32)
    jitter = np.ascontiguousarray(jitter, np.float32)
    W1 = np.asarray(W1, np.float32)
    b1 = np.asarray(b1, np.float32)
    W2 = np.asarray(W2, np.float32)
    b2 = np.asarray(b2, np.float32)

    shared = _shared_inputs(W1, b1, W2, b2)
    in_maps = []
    for k in range(N_CORES):
        xp, dens = _pack_core_inputs(density_grid, jitter, W1, b1, W2, b2, k)
        in_maps.append({"xp": xp, "dens": dens, **shared})

    if "nc" not in _CACHE:
        _CACHE["nc"] = _build_nc()
    nc = _CACHE["nc"]

    trace = bool(int(os.environ.get("DG_TRACE", "0")))
    res = bass_utils.run_bass_kernel_spmd(
        nc, in_maps, core_ids=list(range(N_CORES)), trace=trace)
    LAST_RESULT = res
    outs = res.results

    new_grid = np.empty((CASC, N_CELLS), np.float32)
    bf = np.empty((CASC, N_CELLS // 8), np.uint8)
    for k in range(N_CORES):
        new_grid[:, k * SLAB:(k + 1) * SLAB] = \
            outs[k]["new_out"].reshape(CASC, SLAB)
        bf[:, k * SLAB // 8:(k + 1) * SLAB // 8] = \
            outs[k]["bf_out"].reshape(CASC, SLAB // 8)
    mean_density = outs[0]["mean_out"].reshape(())[()]
    return new_grid, np.float32(mean_density), bf.reshape(-1)
